# revision 6
# baseline (speedup 1.0000x reference)
"""DeepSeek MoE gate (noaux_tc routing) on 8 TRN2 NeuronCores — v2.

Full inputs:
    hidden_states            [8192, 7168] f32
    weight                   [256, 7168]  f32
    e_score_correction_bias  [256]        f32
Full outputs (tuple, like the reference):
    routing_weights          [8192, 8] f32
    selected_experts         [8192, 8] int32

Sharding: token dim split 8 ways (1024 tokens/core); weight + bias replicated.

v2 design (vs v1): host pre-transposes the gate weight (no PE transposes or
PSUM staging for w), x transposes batched 4-chunks-per-PSUM-bank with a single
evacuation copy, matmuls use a 512-wide fused moving stream [wh || wl] so each
k-chunk is one LDWEIGHTS (hidden) + one 512-row stream.

Schemes (MOE_V2 env):
  f32 — full-precision fp32 matmuls, logits exact; 2-pass moving streams
  s2  — xh @ [wh || wl] = round(x) @ w; error from x rounding only (~2^-12)
  s3  — adds xl @ wh correction pass: f32r3-class accuracy (~2^-24)
"""

import os
import sys

import numpy as np

T_FULL = 8192
H = 7168
E = 256
N_CORES = 8
T_LOC = T_FULL // N_CORES          # 1024 tokens per core
P = 128                            # partition tile
N_TTILES = T_LOC // P              # 8 token tiles per core
N_K = H // P                       # 56 contraction chunks
GK = 4                             # k-chunks per transpose/evac group
N_G = N_K // GK                    # 14 groups per token tile
N_GROUP = 8
EG = E // N_GROUP                  # 32 experts per group
TOPK_GROUP = 4
TOP_K = 8
ROUTED_SCALING = 2.5

SCHEME = os.environ.get("MOE_V2", "s3")
assert SCHEME in ("f32", "s2", "s3")


def _ensure_path():
    for p in ("/opt/trn_rl_repo", "/root/.axon_site/_ro/trn_rl_repo"):
        if os.path.isdir(p) and p not in sys.path:
            sys.path.append(p)


def _build_program():
    _ensure_path()
    import concourse.bass as bass  # noqa: F401
    import concourse.mybir as mybir
    from concourse import bacc
    from concourse.tile import TileContext

    f32 = mybir.dt.float32
    f32r = mybir.dt.float32r
    u32 = mybir.dt.uint32
    i32 = mybir.dt.int32
    Alu = mybir.AluOpType
    Act = mybir.ActivationFunctionType

    full_fp32 = SCHEME == "f32"
    residual = SCHEME == "s3"           # extra xl @ wh pass
    mm_dt = f32 if full_fp32 else f32r
    # exact (full f32) transposes needed when xl must be derived from them
    tr_exact = full_fp32 or residual
    WW = E if full_fp32 else 2 * E      # moving-stream width per k-chunk

    nc = bacc.Bacc("TRN2", debug=False, enable_asserts=False)

    hs = nc.dram_tensor("hidden_states", [T_LOC, H], f32, kind="ExternalInput")
    wt = nc.dram_tensor("wT", [H, E], f32, kind="ExternalInput")
    bias = nc.dram_tensor(
        "e_score_correction_bias", [E], f32, kind="ExternalInput"
    )
    out_w = nc.dram_tensor("routing_weights", [T_LOC, TOP_K], f32, kind="ExternalOutput")
    out_i = nc.dram_tensor("selected_experts", [T_LOC, TOP_K], i32, kind="ExternalOutput")

    with TileContext(nc) as tc:
        with (
            tc.tile_pool(name="const", bufs=1) as const_pool,
            tc.tile_pool(name="wsb", bufs=1) as wsb_pool,
            tc.tile_pool(name="x", bufs=3 if residual else 4) as x_pool,
            tc.tile_pool(name="xt", bufs=4) as xt_pool,
            tc.tile_pool(name="ps_t", bufs=3, space="PSUM") as ps_t_pool,
            tc.tile_pool(name="ps_l", bufs=2, space="PSUM") as ps_l_pool,
            tc.tile_pool(name="epi", bufs=2) as epi_pool,
            tc.tile_pool(name="stage", bufs=1) as stage_pool,
        ):
            # ---- constants -------------------------------------------------
            eye_np = np.eye(P, dtype=np.float32)
            eye_dram = nc.inline_tensor(eye_np, name="eye128")
            identity = const_pool.tile([P, P], f32)
            nc.sync.dma_start(out=identity, in_=eye_dram.ap())

            ones_dram = nc.inline_tensor(
                np.ones((1, P), dtype=np.float32), name="ones128"
            )
            ones_row = const_pool.tile([1, P], f32)
            nc.sync.dma_start(out=ones_row, in_=ones_dram.ap())

            bias_row = const_pool.tile([1, E], f32)
            nc.sync.dma_start(
                out=bias_row, in_=bias.ap().rearrange("(o e) -> o e", o=1)
            )
            # broadcast bias across partitions: rank-1 matmul ones^T @ bias_row
            bias_bc = const_pool.tile([P, E], f32)
            ps_b = ps_t_pool.tile([P, E], f32, tag="ps_t")
            nc.tensor.matmul(ps_b, lhsT=ones_row, rhs=bias_row, start=True, stop=True)
            nc.vector.tensor_copy(bias_bc, ps_b)

            # ---- gate weight into SBUF ------------------------------------
            # wsb[:, k, 0:E] = wh chunk k; [:, k, E:2E] = wl (s2/s3 only)
            wsb = wsb_pool.tile([P, N_K, WW], mm_dt)
            N_WDMA = 8
            KC = N_K // N_WDMA
            if full_fp32:
                for q in range(N_WDMA):
                    nc.sync.dma_start(
                        out=wsb[:, q * KC : (q + 1) * KC, :],
                        in_=wt.ap().rearrange("(k p) e -> p k e", p=P)[
                            :, q * KC : (q + 1) * KC, :
                        ],
                    )
            else:
                with tc.tile_pool(name="wnat", bufs=2) as wnat_pool:
                    for q in range(N_WDMA):
                        w_nat = wnat_pool.tile([P, KC, E], f32, tag="wnat")
                        nc.sync.dma_start(
                            out=w_nat,
                            in_=wt.ap().rearrange("(k p) e -> p k e", p=P)[
                                :, q * KC : (q + 1) * KC, :
                            ],
                        )
                        for kk in range(KC):
                            k = q * KC + kk
                            # wh = f32r(w): ACT cast rounds
                            nc.scalar.copy(wsb[:, k, 0:E], w_nat[:, kk, :])
                            # wl = w - wh (exactly representable in f32r)
                            nc.vector.scalar_tensor_tensor(
                                out=wsb[:, k, E : 2 * E],
                                in0=wsb[:, k, 0:E],
                                scalar=-1.0,
                                in1=w_nat[:, kk, :],
                                op0=Alu.mult,
                                op1=Alu.add,
                            )

            # ---- output staging -------------------------------------------
            stage_w = stage_pool.tile([P, N_TTILES, TOP_K], f32)
            stage_i = stage_pool.tile([P, N_TTILES, TOP_K], u32)

            # ---- main loop over token tiles --------------------------------
            for ti in range(N_TTILES):
                x_half = []
                for h in range(2):
                    xha = x_pool.tile([P, H // 2], f32, tag="x", name=f"xh_{ti}_{h}")
                    x_half.append(xha)
                    nc.sync.dma_start(
                        out=xha,
                        in_=hs.ap()[
                            ti * P : (ti + 1) * P,
                            h * (H // 2) : (h + 1) * (H // 2),
                        ],
                    )

                ps_log = ps_l_pool.tile([P, WW], f32, tag="ps_l")

                # software pipeline: transpose+evac group g+1 while group g matmuls
                pend = []
                for g in range(N_G + 1):
                    if g < N_G:
                        pst = ps_t_pool.tile([P, GK * P], f32 if tr_exact else f32r,
                                             tag="ps_t")
                        for j in range(GK):
                            k = g * GK + j
                            src = x_half[k // (N_K // 2)]
                            kk = k % (N_K // 2)
                            blk = src[:, kk * P : (kk + 1) * P]
                            if not tr_exact:
                                blk = blk.bitcast(f32r)
                            nc.tensor.transpose(
                                pst[:, j * P : (j + 1) * P],
                                blk,
                                identity if tr_exact else identity.bitcast(f32r),
                            )
                        xh4 = xt_pool.tile([P, GK * P], mm_dt, tag="xh")
                        # evacuate; alternate engines for balance. ACT cast
                        # rounds to f32r (needed to derive xl in s3).
                        if residual or g % 2 == 0:
                            nc.scalar.copy(xh4, pst)
                        else:
                            nc.vector.tensor_copy(xh4, pst)
                        if residual:
                            xl4 = xt_pool.tile([P, GK * P], mm_dt, tag="xl")
                            nc.vector.scalar_tensor_tensor(
                                out=xl4,
                                in0=xh4,
                                scalar=-1.0,
                                in1=pst,
                                op0=Alu.mult,
                                op1=Alu.add,
                            )
                        else:
                            xl4 = None
                        pend.append((g, xh4, xl4))
                    if g >= 1:
                        g0, xh4, xl4 = pend[g - 1]
                        for j in range(GK):
                            k = g0 * GK + j
                            first = k == 0
                            last = k == N_K - 1
                            xh_j = xh4[:, j * P : (j + 1) * P]
                            nc.tensor.matmul(
                                ps_log, lhsT=xh_j, rhs=wsb[:, k, :],
                                start=first, stop=last and not residual,
                                skip_group_check=residual,
                            )
                            if residual:
                                nc.tensor.matmul(
                                    ps_log[:, 0:E],
                                    lhsT=xl4[:, j * P : (j + 1) * P],
                                    rhs=wsb[:, k, 0:E],
                                    start=False, stop=last,
                                    skip_group_check=True,
                                )

                # ---- epilogue ---------------------------------------------
                scores = epi_pool.tile([P, E], f32, tag="scores")
                if full_fp32:
                    nc.scalar.activation(scores, ps_log, Act.Sigmoid)
                else:
                    logit = epi_pool.tile([P, E], f32, tag="logit")
                    nc.vector.tensor_add(logit, ps_log[:, 0:E], ps_log[:, E : 2 * E])
                    nc.scalar.activation(scores, logit, Act.Sigmoid)

                s_choice = epi_pool.tile([P, E], f32, tag="s_choice")
                nc.vector.tensor_add(s_choice, scores, bias_bc)

                # per-group top-8 (entries 0,1 used) -> group scores
                gmax = epi_pool.tile([P, N_GROUP, 8], f32, tag="gmax")
                for g in range(N_GROUP):
                    nc.vector.max(
                        out=gmax[:, g, :], in_=s_choice[:, g * EG : (g + 1) * EG]
                    )
                gscore = epi_pool.tile([P, N_GROUP], f32, tag="gscore")
                nc.vector.tensor_add(gscore, gmax[:, :, 0], gmax[:, :, 1])

                # top-4 groups: threshold at 4th largest group score
                g8 = epi_pool.tile([P, 8], f32, tag="g8")
                nc.vector.max(out=g8, in_=gscore)
                gmask = epi_pool.tile([P, N_GROUP], f32, tag="gmask")
                nc.vector.tensor_tensor(
                    out=gmask,
                    in0=gscore,
                    in1=g8[:, TOPK_GROUP - 1 : TOPK_GROUP].to_broadcast(
                        [P, N_GROUP]
                    ),
                    op=Alu.is_ge,
                )

                # expand to expert mask and apply
                emask = epi_pool.tile([P, E], f32, tag="emask")
                nc.vector.tensor_copy(
                    emask.rearrange("p (g x) -> p g x", g=N_GROUP),
                    gmask.rearrange("p (g x) -> p g x", x=1).to_broadcast(
                        [P, N_GROUP, EG]
                    ),
                )
                masked = epi_pool.tile([P, E], f32, tag="masked")
                nc.vector.tensor_mul(masked, s_choice, emask)

                # top-8 experts
                v8 = epi_pool.tile([P, 8], f32, tag="v8")
                nc.vector.max(out=v8, in_=masked)
                idx_u = epi_pool.tile([P, 8], u32, tag="idx_u")
                nc.vector.max_index(idx_u, v8, masked)

                # gather raw sigmoid scores at the top-8 positions by matching
                # each top value against the masked tensor (ties have ~0 prob)
                raw8 = epi_pool.tile([P, 8], f32, tag="raw8")
                for kk in range(TOP_K):
                    sc256 = epi_pool.tile([P, E], f32, tag="emask")
                    nc.vector.scalar_tensor_tensor(
                        out=sc256,
                        in0=masked,
                        scalar=v8[:, kk : kk + 1],
                        in1=scores,
                        op0=Alu.is_equal,
                        op1=Alu.mult,
                        accum_out=raw8[:, kk : kk + 1],
                    )

                # normalize * 2.5
                rsum = epi_pool.tile([P, 1], f32, tag="rsum")
                nc.vector.reduce_sum(rsum, raw8, axis=mybir.AxisListType.X)
                nc.vector.tensor_scalar(
                    rsum, rsum, 1.0 / ROUTED_SCALING, None, op0=Alu.mult
                )
                rcp = epi_pool.tile([P, 1], f32, tag="rcp")
                nc.vector.reciprocal(rcp, rsum)
                nc.scalar.mul(stage_w[:, ti, :], raw8, rcp)
                nc.vector.tensor_copy(stage_i[:, ti, :], idx_u)

            # ---- write outputs --------------------------------------------
            nc.sync.dma_start(
                out=out_w.ap().rearrange("(n p) k -> p n k", p=P), in_=stage_w
            )
            nc.sync.dma_start(
                out=out_i.ap().rearrange("(n p) k -> p n k", p=P).bitcast(u32),
                in_=stage_i,
            )

    nc.finalize()
    return nc


_NC_CACHE = {}


def _get_program():
    if SCHEME not in _NC_CACHE:
        _NC_CACHE[SCHEME] = _build_program()
    return _NC_CACHE[SCHEME]


def kernel(hidden_states, weight, e_score_correction_bias, _trace=False):
    _ensure_path()
    from concourse.bass_utils import run_bass_kernel_spmd

    hidden_states = np.ascontiguousarray(hidden_states, dtype=np.float32)
    wT = np.ascontiguousarray(np.asarray(weight, dtype=np.float32).T)
    e_score_correction_bias = np.ascontiguousarray(
        e_score_correction_bias, dtype=np.float32
    )

    nc = _get_program()
    in_maps = [
        {
            "hidden_states": hidden_states[i * T_LOC : (i + 1) * T_LOC],
            "wT": wT,
            "e_score_correction_bias": e_score_correction_bias,
        }
        for i in range(N_CORES)
    ]
    res = run_bass_kernel_spmd(
        nc, in_maps, core_ids=list(range(N_CORES)), trace=_trace
    )
    routing_weights = np.concatenate(
        [res.results[i]["routing_weights"] for i in range(N_CORES)], axis=0
    )
    selected_experts = np.concatenate(
        [res.results[i]["selected_experts"] for i in range(N_CORES)], axis=0
    )
    if _trace:
        return (routing_weights, selected_experts), res
    return routing_weights, selected_experts


# revision 9
# speedup vs baseline: 1.1004x; 1.1004x over previous
"""DeepSeek MoE gate (noaux_tc routing) on 8 TRN2 NeuronCores — v2.

Full inputs:
    hidden_states            [8192, 7168] f32
    weight                   [256, 7168]  f32
    e_score_correction_bias  [256]        f32
Full outputs (tuple, like the reference):
    routing_weights          [8192, 8] f32
    selected_experts         [8192, 8] int32

Sharding: token dim split 8 ways (1024 tokens/core); weight + bias replicated.

v2 design (vs v1): host pre-transposes the gate weight (no PE transposes or
PSUM staging for w), x transposes batched 4-chunks-per-PSUM-bank with a single
evacuation copy, matmuls use a 512-wide fused moving stream [wh || wl] so each
k-chunk is one LDWEIGHTS (hidden) + one 512-row stream.

Schemes (MOE_V2 env):
  f32 — full-precision fp32 matmuls, logits exact; 2-pass moving streams
  s2  — xh @ [wh || wl] = round(x) @ w; error from x rounding only (~2^-12)
  s3  — adds xl @ wh correction pass: f32r3-class accuracy (~2^-24)
"""

import os
import sys

import numpy as np

T_FULL = 8192
H = 7168
E = 256
N_CORES = 8
T_LOC = T_FULL // N_CORES          # 1024 tokens per core
P = 128                            # partition tile
N_TTILES = T_LOC // P              # 8 token tiles per core
N_K = H // P                       # 56 contraction chunks
GK = 4                             # k-chunks per transpose/evac group
N_G = N_K // GK                    # 14 groups per token tile
N_GROUP = 8
EG = E // N_GROUP                  # 32 experts per group
TOPK_GROUP = 4
TOP_K = 8
ROUTED_SCALING = 2.5

SCHEME = os.environ.get("MOE_V2", "s3")
assert SCHEME in ("f32", "s2", "s3")


def _ensure_path():
    for p in ("/opt/trn_rl_repo", "/root/.axon_site/_ro/trn_rl_repo"):
        if os.path.isdir(p) and p not in sys.path:
            sys.path.append(p)


def _build_program():
    _ensure_path()
    import concourse.bass as bass  # noqa: F401
    import concourse.mybir as mybir
    from concourse import bacc
    from concourse.tile import TileContext

    f32 = mybir.dt.float32
    f32r = mybir.dt.float32r
    u32 = mybir.dt.uint32
    i32 = mybir.dt.int32
    Alu = mybir.AluOpType
    Act = mybir.ActivationFunctionType

    full_fp32 = SCHEME == "f32"
    residual = SCHEME == "s3"           # extra xl @ wh pass
    mm_dt = f32 if full_fp32 else f32r
    # exact (full f32) transposes needed when xl must be derived from them
    tr_exact = full_fp32 or residual
    WC = E if full_fp32 else 2 * E      # wsb columns per k-chunk (wh [, wl])

    nc = bacc.Bacc("TRN2", debug=False, enable_asserts=False)

    hs = nc.dram_tensor("hidden_states", [T_LOC, H], f32, kind="ExternalInput")
    wt = nc.dram_tensor("wT", [H, E], f32, kind="ExternalInput")
    bias = nc.dram_tensor(
        "e_score_correction_bias", [E], f32, kind="ExternalInput"
    )
    out_w = nc.dram_tensor("routing_weights", [T_LOC, TOP_K], f32, kind="ExternalOutput")
    out_i = nc.dram_tensor("selected_experts", [T_LOC, TOP_K], i32, kind="ExternalOutput")

    with TileContext(nc) as tc:
        with (
            tc.tile_pool(name="const", bufs=1) as const_pool,
            tc.tile_pool(name="wsb", bufs=1) as wsb_pool,
            tc.tile_pool(name="x", bufs=3 if residual else 4) as x_pool,
            tc.tile_pool(name="xt", bufs=4) as xt_pool,
            tc.tile_pool(name="ps_t", bufs=3, space="PSUM") as ps_t_pool,
            tc.tile_pool(name="ps_l", bufs=2, space="PSUM") as ps_l_pool,
            tc.tile_pool(name="epi", bufs=2) as epi_pool,
            tc.tile_pool(name="stage", bufs=1) as stage_pool,
        ):
            # ---- constants -------------------------------------------------
            eye_np = np.eye(P, dtype=np.float32)
            eye_dram = nc.inline_tensor(eye_np, name="eye128")
            identity = const_pool.tile([P, P], f32)
            nc.sync.dma_start(out=identity, in_=eye_dram.ap())

            ones_dram = nc.inline_tensor(
                np.ones((1, P), dtype=np.float32), name="ones128"
            )
            ones_row = const_pool.tile([1, P], f32)
            nc.sync.dma_start(out=ones_row, in_=ones_dram.ap())

            bias_row = const_pool.tile([1, E], f32)
            nc.sync.dma_start(
                out=bias_row, in_=bias.ap().rearrange("(o e) -> o e", o=1)
            )
            # broadcast bias across partitions: rank-1 matmul ones^T @ bias_row
            bias_bc = const_pool.tile([P, E], f32)
            ps_b = ps_l_pool.tile([P, E], f32, tag="ps_l")
            nc.tensor.matmul(
                ps_b[:, 0:E], lhsT=ones_row, rhs=bias_row, start=True, stop=True
            )
            nc.vector.tensor_copy(bias_bc, ps_b[:, 0:E])

            # ---- gate weight into SBUF ------------------------------------
            # wsb[:, k, 0:E] = wh chunk k; [:, k, E:2E] = wl (s2/s3 only)
            wsb = wsb_pool.tile([P, N_K, WC], mm_dt)
            N_WDMA = 8
            KC = N_K // N_WDMA
            if full_fp32:
                for q in range(N_WDMA):
                    nc.sync.dma_start(
                        out=wsb[:, q * KC : (q + 1) * KC, :],
                        in_=wt.ap().rearrange("(k p) e -> p k e", p=P)[
                            :, q * KC : (q + 1) * KC, :
                        ],
                    )
            else:
                with tc.tile_pool(name="wnat", bufs=2) as wnat_pool:
                    for q in range(N_WDMA):
                        w_nat = wnat_pool.tile([P, KC, E], f32, tag="wnat")
                        nc.sync.dma_start(
                            out=w_nat,
                            in_=wt.ap().rearrange("(k p) e -> p k e", p=P)[
                                :, q * KC : (q + 1) * KC, :
                            ],
                        )
                        for kk in range(KC):
                            k = q * KC + kk
                            # wh = f32r(w): ACT cast rounds
                            nc.scalar.copy(wsb[:, k, 0:E], w_nat[:, kk, :])
                            # wl = w - wh (exactly representable in f32r)
                            nc.vector.scalar_tensor_tensor(
                                out=wsb[:, k, E : 2 * E],
                                in0=wsb[:, k, 0:E],
                                scalar=-1.0,
                                in1=w_nat[:, kk, :],
                                op0=Alu.mult,
                                op1=Alu.add,
                            )

            # ---- output staging -------------------------------------------
            stage_w = stage_pool.tile([P, N_TTILES, TOP_K], f32)
            stage_i = stage_pool.tile([P, N_TTILES, TOP_K], u32)

            # ---- main loop over token tiles --------------------------------
            for ti in range(N_TTILES):
                x_half = []
                for h in range(2):
                    xha = x_pool.tile([P, H // 2], f32, tag="x", name=f"xh_{ti}_{h}")
                    x_half.append(xha)
                    nc.sync.dma_start(
                        out=xha,
                        in_=hs.ap()[
                            ti * P : (ti + 1) * P,
                            h * (H // 2) : (h + 1) * (H // 2),
                        ],
                    )

                ps_log = ps_l_pool.tile([P, E], f32, tag="ps_l")

                # software pipeline: transpose+evac group g+1 while group g matmuls
                pend = []
                for g in range(N_G + 1):
                    if g < N_G:
                        pst = ps_t_pool.tile([P, GK * P], f32 if tr_exact else f32r,
                                             tag="ps_t")
                        for j in range(GK):
                            k = g * GK + j
                            src = x_half[k // (N_K // 2)]
                            kk = k % (N_K // 2)
                            blk = src[:, kk * P : (kk + 1) * P]
                            if not tr_exact:
                                blk = blk.bitcast(f32r)
                            nc.tensor.transpose(
                                pst[:, j * P : (j + 1) * P],
                                blk,
                                identity if tr_exact else identity.bitcast(f32r),
                            )
                        xh4 = xt_pool.tile([P, GK * P], mm_dt, tag="xh")
                        # evacuate; alternate engines for balance. ACT cast
                        # rounds to f32r (needed to derive xl in s3).
                        if residual or g % 2 == 0:
                            nc.scalar.copy(xh4, pst)
                        else:
                            nc.vector.tensor_copy(xh4, pst)
                        if residual:
                            xl4 = xt_pool.tile([P, GK * P], mm_dt, tag="xl")
                            nc.vector.scalar_tensor_tensor(
                                out=xl4,
                                in0=xh4,
                                scalar=-1.0,
                                in1=pst,
                                op0=Alu.mult,
                                op1=Alu.add,
                            )
                        else:
                            xl4 = None
                        pend.append((g, xh4, xl4))
                    if g >= 1:
                        g0, xh4, xl4 = pend[g - 1]
                        for j in range(GK):
                            k = g0 * GK + j
                            first = k == 0
                            last = k == N_K - 1
                            xh_j = xh4[:, j * P : (j + 1) * P]

                            if full_fp32:
                                nc.tensor.matmul(
                                    ps_log, lhsT=xh_j, rhs=wsb[:, k, :],
                                    start=first, stop=last,
                                )
                            else:
                                # xh@wh, xh@wl (+ xl@wh) all accumulate into
                                # the same PSUM region: one chain, no epilogue
                                # recombination needed
                                nc.tensor.matmul(
                                    ps_log, lhsT=xh_j, rhs=wsb[:, k, 0:E],
                                    start=first, stop=False,
                                    skip_group_check=True,
                                )
                                if residual:
                                    nc.tensor.matmul(
                                        ps_log,
                                        lhsT=xl4[:, j * P : (j + 1) * P],
                                        rhs=wsb[:, k, 0:E],
                                        start=False, stop=False,
                                        skip_group_check=True,
                                    )
                                nc.tensor.matmul(
                                    ps_log, lhsT=xh_j, rhs=wsb[:, k, E : 2 * E],
                                    start=False, stop=last,
                                    skip_group_check=True,
                                )

                # ---- epilogue ---------------------------------------------
                scores = epi_pool.tile([P, E], f32, tag="scores")
                nc.scalar.activation(scores, ps_log, Act.Sigmoid)

                s_choice = epi_pool.tile([P, E], f32, tag="s_choice")
                nc.vector.tensor_add(s_choice, scores, bias_bc)

                # per-group top-8 (entries 0,1 used) -> group scores
                gmax = epi_pool.tile([P, N_GROUP, 8], f32, tag="gmax")
                for g in range(N_GROUP):
                    nc.vector.max(
                        out=gmax[:, g, :], in_=s_choice[:, g * EG : (g + 1) * EG]
                    )
                gscore = epi_pool.tile([P, N_GROUP], f32, tag="gscore")
                nc.vector.tensor_add(gscore, gmax[:, :, 0], gmax[:, :, 1])

                # top-4 groups: threshold at 4th largest group score
                g8 = epi_pool.tile([P, 8], f32, tag="g8")
                nc.vector.max(out=g8, in_=gscore)
                gmask = epi_pool.tile([P, N_GROUP], f32, tag="gmask")
                nc.vector.tensor_tensor(
                    out=gmask,
                    in0=gscore,
                    in1=g8[:, TOPK_GROUP - 1 : TOPK_GROUP].to_broadcast(
                        [P, N_GROUP]
                    ),
                    op=Alu.is_ge,
                )

                # expand to expert mask and apply
                emask = epi_pool.tile([P, E], f32, tag="emask")
                nc.vector.tensor_copy(
                    emask.rearrange("p (g x) -> p g x", g=N_GROUP),
                    gmask.rearrange("p (g x) -> p g x", x=1).to_broadcast(
                        [P, N_GROUP, EG]
                    ),
                )
                masked = epi_pool.tile([P, E], f32, tag="masked")
                nc.vector.tensor_mul(masked, s_choice, emask)

                # top-8 experts
                v8 = epi_pool.tile([P, 8], f32, tag="v8")
                nc.vector.max(out=v8, in_=masked)
                idx_u = epi_pool.tile([P, 8], u32, tag="idx_u")
                nc.vector.max_index(idx_u, v8, masked)

                # gather raw sigmoid scores at the top-8 positions by matching
                # each top value against the masked tensor (ties have ~0 prob)
                raw8 = epi_pool.tile([P, 8], f32, tag="raw8")
                for kk in range(TOP_K):
                    sc256 = epi_pool.tile([P, E], f32, tag="emask")
                    nc.vector.scalar_tensor_tensor(
                        out=sc256,
                        in0=masked,
                        scalar=v8[:, kk : kk + 1],
                        in1=scores,
                        op0=Alu.is_equal,
                        op1=Alu.mult,
                        accum_out=raw8[:, kk : kk + 1],
                    )

                # normalize * 2.5
                rsum = epi_pool.tile([P, 1], f32, tag="rsum")
                nc.vector.reduce_sum(rsum, raw8, axis=mybir.AxisListType.X)
                nc.vector.tensor_scalar(
                    rsum, rsum, 1.0 / ROUTED_SCALING, None, op0=Alu.mult
                )
                rcp = epi_pool.tile([P, 1], f32, tag="rcp")
                nc.vector.reciprocal(rcp, rsum)
                nc.scalar.mul(stage_w[:, ti, :], raw8, rcp)
                nc.vector.tensor_copy(stage_i[:, ti, :], idx_u)

            # ---- write outputs --------------------------------------------
            nc.sync.dma_start(
                out=out_w.ap().rearrange("(n p) k -> p n k", p=P), in_=stage_w
            )
            nc.sync.dma_start(
                out=out_i.ap().rearrange("(n p) k -> p n k", p=P).bitcast(u32),
                in_=stage_i,
            )

    nc.finalize()
    return nc


_NC_CACHE = {}


def _get_program():
    if SCHEME not in _NC_CACHE:
        _NC_CACHE[SCHEME] = _build_program()
    return _NC_CACHE[SCHEME]


def kernel(hidden_states, weight, e_score_correction_bias, _trace=False):
    _ensure_path()
    from concourse.bass_utils import run_bass_kernel_spmd

    hidden_states = np.ascontiguousarray(hidden_states, dtype=np.float32)
    wT = np.ascontiguousarray(np.asarray(weight, dtype=np.float32).T)
    e_score_correction_bias = np.ascontiguousarray(
        e_score_correction_bias, dtype=np.float32
    )

    nc = _get_program()
    in_maps = [
        {
            "hidden_states": hidden_states[i * T_LOC : (i + 1) * T_LOC],
            "wT": wT,
            "e_score_correction_bias": e_score_correction_bias,
        }
        for i in range(N_CORES)
    ]
    res = run_bass_kernel_spmd(
        nc, in_maps, core_ids=list(range(N_CORES)), trace=_trace
    )
    routing_weights = np.concatenate(
        [res.results[i]["routing_weights"] for i in range(N_CORES)], axis=0
    )
    selected_experts = np.concatenate(
        [res.results[i]["selected_experts"] for i in range(N_CORES)], axis=0
    )
    if _trace:
        return (routing_weights, selected_experts), res
    return routing_weights, selected_experts


# revision 11
# speedup vs baseline: 1.3963x; 1.2690x over previous
"""DeepSeek MoE gate (noaux_tc routing) on 8 TRN2 NeuronCores — v2.

Full inputs:
    hidden_states            [8192, 7168] f32
    weight                   [256, 7168]  f32
    e_score_correction_bias  [256]        f32
Full outputs (tuple, like the reference):
    routing_weights          [8192, 8] f32
    selected_experts         [8192, 8] int32

Sharding: token dim split 8 ways (1024 tokens/core); weight + bias replicated.

v2 design (vs v1): host pre-transposes the gate weight (no PE transposes or
PSUM staging for w), x transposes batched 4-chunks-per-PSUM-bank with a single
evacuation copy, matmuls use a 512-wide fused moving stream [wh || wl] so each
k-chunk is one LDWEIGHTS (hidden) + one 512-row stream.

Schemes (MOE_V2 env):
  f32 — full-precision fp32 matmuls, logits exact; 2-pass moving streams
  s2  — xh @ [wh || wl] = round(x) @ w; error from x rounding only (~2^-12)
  s3  — adds xl @ wh correction pass: f32r3-class accuracy (~2^-24)
"""

import os
import sys

import numpy as np

T_FULL = 8192
H = 7168
E = 256
N_CORES = 8
T_LOC = T_FULL // N_CORES          # 1024 tokens per core
P = 128                            # partition tile
N_TTILES = T_LOC // P              # 8 token tiles per core
N_K = H // P                       # 56 contraction chunks
GK = 4                             # k-chunks per transpose/evac group
N_G = N_K // GK                    # 14 groups per token tile
N_GROUP = 8
EG = E // N_GROUP                  # 32 experts per group
TOPK_GROUP = 4
TOP_K = 8
ROUTED_SCALING = 2.5

SCHEME = os.environ.get("MOE_V2", "s3")
assert SCHEME in ("f32", "s2", "s3")


def _ensure_path():
    for p in ("/opt/trn_rl_repo", "/root/.axon_site/_ro/trn_rl_repo"):
        if os.path.isdir(p) and p not in sys.path:
            sys.path.append(p)


def _build_program():
    _ensure_path()
    import concourse.bass as bass  # noqa: F401
    import concourse.mybir as mybir
    from concourse import bacc
    from concourse.tile import TileContext

    f32 = mybir.dt.float32
    f32r = mybir.dt.float32r
    u32 = mybir.dt.uint32
    i32 = mybir.dt.int32
    Alu = mybir.AluOpType
    Act = mybir.ActivationFunctionType

    full_fp32 = SCHEME == "f32"
    residual = SCHEME == "s3"           # extra xl @ wh pass
    mm_dt = f32 if full_fp32 else f32r
    # transposes stay full fp32: the f32r cast must happen in a rounding
    # instruction (ACT/DVE copy) per the BIR verifier, not via bitcast
    tr_exact = True
    WC = E if full_fp32 else 2 * E      # wsb columns per k-chunk (wh [, wl])

    nc = bacc.Bacc("TRN2", debug=False, enable_asserts=False)

    hs = nc.dram_tensor("hidden_states", [T_LOC, H], f32, kind="ExternalInput")
    wt = nc.dram_tensor("wT", [H, E], f32, kind="ExternalInput")
    bias = nc.dram_tensor(
        "e_score_correction_bias", [E], f32, kind="ExternalInput"
    )
    out_w = nc.dram_tensor("routing_weights", [T_LOC, TOP_K], f32, kind="ExternalOutput")
    out_i = nc.dram_tensor("selected_experts", [T_LOC, TOP_K], i32, kind="ExternalOutput")

    with TileContext(nc) as tc:
        with (
            tc.tile_pool(name="const", bufs=1) as const_pool,
            tc.tile_pool(name="wsb", bufs=1) as wsb_pool,
            tc.tile_pool(name="x", bufs=3 if residual else 4) as x_pool,
            tc.tile_pool(name="xt", bufs=4) as xt_pool,
            tc.tile_pool(name="ps_t", bufs=3, space="PSUM") as ps_t_pool,
            tc.tile_pool(name="ps_l", bufs=2, space="PSUM") as ps_l_pool,
            tc.tile_pool(name="epi", bufs=2) as epi_pool,
            tc.tile_pool(name="stage", bufs=1) as stage_pool,
        ):
            # ---- constants -------------------------------------------------
            eye_np = np.eye(P, dtype=np.float32)
            eye_dram = nc.inline_tensor(eye_np, name="eye128")
            identity = const_pool.tile([P, P], f32)
            nc.sync.dma_start(out=identity, in_=eye_dram.ap())

            ones_dram = nc.inline_tensor(
                np.ones((1, P), dtype=np.float32), name="ones128"
            )
            ones_row = const_pool.tile([1, P], f32)
            nc.sync.dma_start(out=ones_row, in_=ones_dram.ap())

            bias_row = const_pool.tile([1, E], f32)
            nc.sync.dma_start(
                out=bias_row, in_=bias.ap().rearrange("(o e) -> o e", o=1)
            )
            # broadcast bias across partitions: rank-1 matmul ones^T @ bias_row
            bias_bc = const_pool.tile([P, E], f32)
            ps_b = ps_l_pool.tile([P, WC], f32, tag="ps_l")
            nc.tensor.matmul(
                ps_b[:, 0:E], lhsT=ones_row, rhs=bias_row, start=True, stop=True
            )
            nc.vector.tensor_copy(bias_bc, ps_b[:, 0:E])

            # ---- gate weight into SBUF ------------------------------------
            # wsb[:, k, 0:E] = wh chunk k; [:, k, E:2E] = wl (s2/s3 only)
            wsb = wsb_pool.tile([P, N_K, WC], mm_dt)
            N_WDMA = 8
            KC = N_K // N_WDMA
            if full_fp32:
                for q in range(N_WDMA):
                    nc.sync.dma_start(
                        out=wsb[:, q * KC : (q + 1) * KC, :],
                        in_=wt.ap().rearrange("(k p) e -> p k e", p=P)[
                            :, q * KC : (q + 1) * KC, :
                        ],
                    )
            else:
                with tc.tile_pool(name="wnat", bufs=2) as wnat_pool:
                    for q in range(N_WDMA):
                        w_nat = wnat_pool.tile([P, KC, E], f32, tag="wnat")
                        nc.sync.dma_start(
                            out=w_nat,
                            in_=wt.ap().rearrange("(k p) e -> p k e", p=P)[
                                :, q * KC : (q + 1) * KC, :
                            ],
                        )
                        for kk in range(KC):
                            k = q * KC + kk
                            # wh = f32r(w): ACT cast rounds
                            nc.scalar.copy(wsb[:, k, 0:E], w_nat[:, kk, :])
                            # wl = w - wh (exactly representable in f32r)
                            nc.vector.scalar_tensor_tensor(
                                out=wsb[:, k, E : 2 * E],
                                in0=wsb[:, k, 0:E],
                                scalar=-1.0,
                                in1=w_nat[:, kk, :],
                                op0=Alu.mult,
                                op1=Alu.add,
                            )

            # ---- output staging -------------------------------------------
            stage_w = stage_pool.tile([P, N_TTILES, TOP_K], f32)
            stage_i = stage_pool.tile([P, N_TTILES, TOP_K], u32)

            # ---- main loop over token tiles --------------------------------
            for ti in range(N_TTILES):
                x_half = []
                for h in range(2):
                    xha = x_pool.tile([P, H // 2], f32, tag="x", name=f"xh_{ti}_{h}")
                    x_half.append(xha)
                    nc.sync.dma_start(
                        out=xha,
                        in_=hs.ap()[
                            ti * P : (ti + 1) * P,
                            h * (H // 2) : (h + 1) * (H // 2),
                        ],
                    )

                ps_log = ps_l_pool.tile([P, WC], f32, tag="ps_l")

                # software pipeline: transpose+evac group g+1 while group g matmuls
                pend = []
                for g in range(N_G + 1):
                    if g < N_G:
                        pst = ps_t_pool.tile([P, GK * P], f32 if tr_exact else f32r,
                                             tag="ps_t")
                        for j in range(GK):
                            k = g * GK + j
                            src = x_half[k // (N_K // 2)]
                            kk = k % (N_K // 2)
                            blk = src[:, kk * P : (kk + 1) * P]
                            if not tr_exact:
                                blk = blk.bitcast(f32r)
                            nc.tensor.transpose(
                                pst[:, j * P : (j + 1) * P],
                                blk,
                                identity if tr_exact else identity.bitcast(f32r),
                            )
                        xh4 = xt_pool.tile([P, GK * P], mm_dt, tag="xh")
                        # evacuate; alternate engines for balance. ACT cast
                        # rounds to f32r (needed to derive xl in s3).
                        if residual or g % 2 == 0:
                            nc.scalar.copy(xh4, pst)
                        else:
                            nc.vector.tensor_copy(xh4, pst)
                        if residual:
                            xl4 = xt_pool.tile([P, GK * P], mm_dt, tag="xl")
                            nc.vector.scalar_tensor_tensor(
                                out=xl4,
                                in0=xh4,
                                scalar=-1.0,
                                in1=pst,
                                op0=Alu.mult,
                                op1=Alu.add,
                            )
                        else:
                            xl4 = None
                        pend.append((g, xh4, xl4))
                    if g >= 1:
                        g0, xh4, xl4 = pend[g - 1]
                        for j in range(GK):
                            k = g0 * GK + j
                            first = k == 0
                            last = k == N_K - 1
                            xh_j = xh4[:, j * P : (j + 1) * P]

                            if full_fp32:
                                nc.tensor.matmul(
                                    ps_log, lhsT=xh_j, rhs=wsb[:, k, :],
                                    start=first, stop=last,
                                )
                            elif residual:
                                # fused xh @ [wh || wl] plus xl @ wh into the
                                # low half; last matmul closes the full bank
                                if not last:
                                    nc.tensor.matmul(
                                        ps_log, lhsT=xh_j, rhs=wsb[:, k, :],
                                        start=first, stop=False,
                                        skip_group_check=True,
                                    )
                                    nc.tensor.matmul(
                                        ps_log[:, 0:E],
                                        lhsT=xl4[:, j * P : (j + 1) * P],
                                        rhs=wsb[:, k, 0:E],
                                        start=False, stop=False,
                                        skip_group_check=True,
                                    )
                                else:
                                    nc.tensor.matmul(
                                        ps_log[:, 0:E],
                                        lhsT=xl4[:, j * P : (j + 1) * P],
                                        rhs=wsb[:, k, 0:E],
                                        start=False, stop=False,
                                        skip_group_check=True,
                                    )
                                    nc.tensor.matmul(
                                        ps_log, lhsT=xh_j, rhs=wsb[:, k, :],
                                        start=False, stop=True,
                                        skip_group_check=True,
                                    )
                            else:
                                nc.tensor.matmul(
                                    ps_log, lhsT=xh_j, rhs=wsb[:, k, :],
                                    start=first, stop=last,
                                )

                # ---- epilogue ---------------------------------------------
                scores = epi_pool.tile([P, E], f32, tag="scores")
                if full_fp32:
                    nc.scalar.activation(scores, ps_log, Act.Sigmoid)
                else:
                    hi = epi_pool.tile([P, E], f32, tag="hi")
                    nc.scalar.copy(hi, ps_log[:, E : 2 * E])
                    logit = epi_pool.tile([P, E], f32, tag="logit")
                    nc.vector.tensor_add(logit, ps_log[:, 0:E], hi)
                    nc.scalar.activation(scores, logit, Act.Sigmoid)

                s_choice = epi_pool.tile([P, E], f32, tag="s_choice")
                nc.vector.tensor_add(s_choice, scores, bias_bc)

                # per-group top-8 (entries 0,1 used) -> group scores
                gmax = epi_pool.tile([P, N_GROUP, 8], f32, tag="gmax")
                for g in range(N_GROUP):
                    nc.vector.max(
                        out=gmax[:, g, :], in_=s_choice[:, g * EG : (g + 1) * EG]
                    )
                gscore = epi_pool.tile([P, N_GROUP], f32, tag="gscore")
                nc.vector.tensor_add(gscore, gmax[:, :, 0], gmax[:, :, 1])

                # top-4 groups: threshold at 4th largest group score
                g8 = epi_pool.tile([P, 8], f32, tag="g8")
                nc.vector.max(out=g8, in_=gscore)
                gmask = epi_pool.tile([P, N_GROUP], f32, tag="gmask")
                nc.vector.tensor_tensor(
                    out=gmask,
                    in0=gscore,
                    in1=g8[:, TOPK_GROUP - 1 : TOPK_GROUP].to_broadcast(
                        [P, N_GROUP]
                    ),
                    op=Alu.is_ge,
                )

                # expand to expert mask and apply
                emask = epi_pool.tile([P, E], f32, tag="emask")
                nc.vector.tensor_copy(
                    emask.rearrange("p (g x) -> p g x", g=N_GROUP),
                    gmask.rearrange("p (g x) -> p g x", x=1).to_broadcast(
                        [P, N_GROUP, EG]
                    ),
                )
                masked = epi_pool.tile([P, E], f32, tag="masked")
                nc.vector.tensor_mul(masked, s_choice, emask)

                # top-8 experts
                v8 = epi_pool.tile([P, 8], f32, tag="v8")
                nc.vector.max(out=v8, in_=masked)
                idx_u = epi_pool.tile([P, 8], u32, tag="idx_u")
                nc.vector.max_index(idx_u, v8, masked)

                # gather raw sigmoid scores at the top-8 positions by matching
                # each top value against the masked tensor (ties have ~0 prob)
                raw8 = epi_pool.tile([P, 8], f32, tag="raw8")
                for kk in range(TOP_K):
                    sc256 = epi_pool.tile([P, E], f32, tag="emask")
                    nc.vector.scalar_tensor_tensor(
                        out=sc256,
                        in0=masked,
                        scalar=v8[:, kk : kk + 1],
                        in1=scores,
                        op0=Alu.is_equal,
                        op1=Alu.mult,
                        accum_out=raw8[:, kk : kk + 1],
                    )

                # normalize * 2.5
                rsum = epi_pool.tile([P, 1], f32, tag="rsum")
                nc.vector.reduce_sum(rsum, raw8, axis=mybir.AxisListType.X)
                nc.vector.tensor_scalar(
                    rsum, rsum, 1.0 / ROUTED_SCALING, None, op0=Alu.mult
                )
                rcp = epi_pool.tile([P, 1], f32, tag="rcp")
                nc.vector.reciprocal(rcp, rsum)
                nc.scalar.mul(stage_w[:, ti, :], raw8, rcp)
                nc.vector.tensor_copy(stage_i[:, ti, :], idx_u)

            # ---- write outputs --------------------------------------------
            nc.sync.dma_start(
                out=out_w.ap().rearrange("(n p) k -> p n k", p=P), in_=stage_w
            )
            nc.sync.dma_start(
                out=out_i.ap().rearrange("(n p) k -> p n k", p=P).bitcast(u32),
                in_=stage_i,
            )

    nc.finalize()
    return nc


_NC_CACHE = {}


def _get_program():
    if SCHEME not in _NC_CACHE:
        _NC_CACHE[SCHEME] = _build_program()
    return _NC_CACHE[SCHEME]


def kernel(hidden_states, weight, e_score_correction_bias, _trace=False):
    _ensure_path()
    from concourse.bass_utils import run_bass_kernel_spmd

    hidden_states = np.ascontiguousarray(hidden_states, dtype=np.float32)
    wT = np.ascontiguousarray(np.asarray(weight, dtype=np.float32).T)
    e_score_correction_bias = np.ascontiguousarray(
        e_score_correction_bias, dtype=np.float32
    )

    nc = _get_program()
    in_maps = [
        {
            "hidden_states": hidden_states[i * T_LOC : (i + 1) * T_LOC],
            "wT": wT,
            "e_score_correction_bias": e_score_correction_bias,
        }
        for i in range(N_CORES)
    ]
    res = run_bass_kernel_spmd(
        nc, in_maps, core_ids=list(range(N_CORES)), trace=_trace
    )
    routing_weights = np.concatenate(
        [res.results[i]["routing_weights"] for i in range(N_CORES)], axis=0
    )
    selected_experts = np.concatenate(
        [res.results[i]["selected_experts"] for i in range(N_CORES)], axis=0
    )
    if _trace:
        return (routing_weights, selected_experts), res
    return routing_weights, selected_experts
